# revision 9
# baseline (speedup 1.0000x reference)
"""ArcMargin softmax-with-loss on 8 TRN2 NeuronCores (Bass/Tile).

Strategy (model-parallel softmax cross-entropy):
  - Shard num_classes (axis 1) across 8 cores: each core holds a
    [512, 12500] f32 slice of cos_theta.
  - Since S*cos_theta is bounded by S=30, use a FIXED logsumexp shift of
    30 (exp(S*x-30) in (e^-60, 1]) -> no rowwise-max all-reduce needed.
  - Each core streams its 25.6MB shard once: ScalarE activation
    exp(S*x - 30) with accum_out produces rowwise partial sums fused
    with the elementwise pass (DMA-bound, ~358 GB/s/core roofline).
  - The target column's margin correction: gather x_t = cos[b, t_b] via
    indirect DMA on the owning core, compute
    phi = where(x > TH, x*cos(M) - sqrt(1-x^2)*sin(M), x - MM),
    corr = exp(S*phi-30) - exp(S*x_t-30), tgt = S*phi; mask to owner.
  - One 8KB AllReduce(add) combines [Z_partial + corr, tgt] across
    cores; every core then computes loss = mean(log(Z') + 30 - tgt).

GAMMA=0 in the reference makes (1-pt)^GAMMA == 1, so
loss = mean_b(logsumexp_c(out[b,:]) - out[b, t_b]).
"""

import math

import numpy as np

import concourse.bacc as bacc
import concourse.bass as bass
import concourse.tile as tile
from concourse import mybir
from concourse import bass_utils

S = 30.0
M = 0.5
COS_M = math.cos(M)
SIN_M = math.sin(M)
TH = math.cos(math.pi - M)
MM = math.sin(math.pi - M) * M
SHIFT = 30.0  # fixed logsumexp shift; S*cos_theta <= 30

N_CORES = 8
B = 512
C = 100000
C_LOC = C // N_CORES  # 12500
P = 128
NBLK = B // P  # 4 row blocks of 128
F32 = mybir.dt.float32
I32 = mybir.dt.int32
AF = mybir.ActivationFunctionType
ALU = mybir.AluOpType


def build(c_loc=C_LOC, chunk=3125, stream_bufs=4):
    assert c_loc % chunk == 0
    nch = c_loc // chunk

    nc = bacc.Bacc(
        "TRN2", target_bir_lowering=False, debug=False, num_devices=N_CORES
    )
    x = nc.dram_tensor("x", [B * c_loc], F32, kind="ExternalInput")
    # lcoff[p, k*nch+j] = target's column within chunk j of block k's row
    # (k*128+p) if this core owns that row's target, else -1 (matches no
    # iota value -> one-hot mask is all-zero).
    lcoff = nc.dram_tensor("lcoff", [P, NBLK * nch], I32, kind="ExternalInput")
    mask = nc.dram_tensor("mask", [P, NBLK], F32, kind="ExternalInput")
    out = nc.dram_tensor("out", [1, 1], F32, kind="ExternalOutput")

    x3 = x.ap().rearrange("(k p c) -> k p c", p=P, c=c_loc)  # [NBLK, P, c_loc]

    with tile.TileContext(nc) as tc:
        with (
            tc.tile_pool(name="stream", bufs=stream_bufs) as stream,
            tc.tile_pool(name="mscratch", bufs=2) as mscratch,
            tc.tile_pool(name="small", bufs=1) as small,
            tc.tile_pool(name="dram", bufs=1, space="DRAM") as dram,
            tc.tile_pool(name="psum", bufs=1, space="PSUM") as psum,
        ):
            lcoff_sb = small.tile([P, NBLK * nch], I32)
            mask_sb = small.tile([P, NBLK], F32)
            nc.sync.dma_start(out=lcoff_sb[:], in_=lcoff.ap())
            nc.sync.dma_start(out=mask_sb[:], in_=mask.ap())

            nbias = small.tile([P, 1], F32)  # bias AP = -SHIFT for Exp calls
            nc.vector.memset(nbias[:], -SHIFT)

            iota_sb = small.tile([P, chunk], I32)
            nc.gpsimd.iota(
                iota_sb[:], pattern=[[1, chunk]], base=0, channel_multiplier=0
            )

            # --- streaming pass ------------------------------------------
            # Per chunk: ACT computes exp(S*x-30) in-place with rowwise
            # accum (partial Z); DVE extracts the target element via a
            # fused one-hot masked reduce: (iota == lcoff) * x, accum.
            acc = small.tile([P, NBLK * nch], F32)
            xacc = small.tile([P, NBLK * nch], F32)
            for k in range(NBLK):
                for j in range(nch):
                    t = stream.tile([P, chunk], F32, tag="stream")
                    nc.sync.dma_start(
                        out=t[:], in_=x3[k, :, j * chunk : (j + 1) * chunk]
                    )
                    col = k * nch + j
                    m = mscratch.tile([P, chunk], F32, tag="m")
                    nc.vector.scalar_tensor_tensor(
                        out=m[:],
                        in0=iota_sb[:],
                        scalar=lcoff_sb[:, col : col + 1],
                        in1=t[:],
                        op0=ALU.is_equal,
                        op1=ALU.mult,
                        accum_out=xacc[:, col : col + 1],
                    )
                    nc.scalar.activation(
                        t[:],
                        t[:],
                        AF.Exp,
                        bias=nbias[:],
                        scale=S,
                        accum_out=acc[:, col : col + 1],
                    )

            zp = small.tile([P, NBLK], F32)
            nc.vector.tensor_reduce(
                zp[:],
                acc[:].rearrange("p (k j) -> p k j", j=nch),
                axis=mybir.AxisListType.X,
                op=ALU.add,
            )
            xg = small.tile([P, NBLK], F32)
            nc.vector.tensor_reduce(
                xg[:],
                xacc[:].rearrange("p (k j) -> p k j", j=nch),
                axis=mybir.AxisListType.X,
                op=ALU.add,
            )

            # sin = sqrt(relu(1 - x^2)) computed as exp(0.5*ln(.)) to stay
            # within the natural_log_exp ACT table set (no sqrt set load).
            t1 = small.tile([P, NBLK], F32)
            nc.vector.tensor_mul(t1[:], xg[:], xg[:])
            nc.vector.tensor_scalar(t1[:], t1[:], -1.0, 1.0, ALU.mult, ALU.add)
            nc.vector.tensor_scalar_max(t1[:], t1[:], 0.0)
            nc.scalar.activation(t1[:], t1[:], AF.Ln)
            nc.scalar.activation(t1[:], t1[:], AF.Exp, scale=0.5)
            phi = small.tile([P, NBLK], F32)
            nc.vector.tensor_scalar(t1[:], t1[:], SIN_M, None, ALU.mult)
            nc.vector.tensor_scalar(phi[:], xg[:], COS_M, None, ALU.mult)
            nc.vector.tensor_sub(phi[:], phi[:], t1[:])
            alt = small.tile([P, NBLK], F32)
            nc.vector.tensor_scalar(alt[:], xg[:], -MM, None, ALU.add)
            cond = small.tile([P, NBLK], I32)  # CopyPredicated needs int mask
            nc.vector.tensor_scalar(cond[:], xg[:], TH, None, ALU.is_le)
            nc.vector.copy_predicated(phi[:], cond[:], alt[:])

            e1 = small.tile([P, NBLK], F32)
            e2 = small.tile([P, NBLK], F32)
            nc.scalar.activation(e1[:], phi[:], AF.Exp, bias=nbias[:], scale=S)
            nc.scalar.activation(e2[:], xg[:], AF.Exp, bias=nbias[:], scale=S)
            nc.vector.tensor_sub(e1[:], e1[:], e2[:])
            nc.vector.tensor_mul(e1[:], e1[:], mask_sb[:])  # corr
            tgt = small.tile([P, NBLK], F32)
            nc.vector.tensor_scalar(tgt[:], phi[:], S, None, ALU.mult)
            nc.vector.tensor_mul(tgt[:], tgt[:], mask_sb[:])

            # --- all-reduce [Z_partial + corr | tgt] ----------------------
            ar_sb = small.tile([P, 2 * NBLK], F32)
            nc.vector.tensor_add(ar_sb[:, 0:NBLK], zp[:], e1[:])
            nc.vector.tensor_copy(ar_sb[:, NBLK : 2 * NBLK], tgt[:])

            cc_in = dram.tile([P, 2 * NBLK], F32)
            cc_out = dram.tile([P, 2 * NBLK], F32)
            nc.sync.dma_start(out=cc_in[:], in_=ar_sb[:])
            nc.gpsimd.collective_compute(
                "AllReduce",
                ALU.add,
                replica_groups=[list(range(N_CORES))],
                ins=[cc_in.opt()],
                outs=[cc_out.opt()],
            )
            g = small.tile([P, 2 * NBLK], F32)
            nc.sync.dma_start(out=g[:], in_=cc_out[:])

            # --- loss = mean(log(Z') + SHIFT - tgt) -----------------------
            lg = small.tile([P, NBLK], F32)
            nc.scalar.activation(lg[:], g[:, 0:NBLK], AF.Ln)
            nc.vector.tensor_sub(lg[:], lg[:], g[:, NBLK : 2 * NBLK])
            r1 = small.tile([P, 1], F32)
            nc.vector.tensor_reduce(
                r1[:], lg[:], axis=mybir.AxisListType.X, op=ALU.add
            )
            ones = small.tile([P, 1], F32)
            nc.vector.memset(ones[:], 1.0)
            ps = psum.tile([1, 1], F32)
            nc.tensor.matmul(ps[:], lhsT=r1[:], rhs=ones[:], start=True, stop=True)
            loss = small.tile([1, 1], F32)
            nc.vector.tensor_scalar(
                loss[:], ps[:], 1.0 / B, SHIFT, ALU.mult, ALU.add
            )
            nc.sync.dma_start(out=out.ap(), in_=loss[:])
    nc.finalize()
    return nc


def prep_in_maps(cos_theta, target, c_loc=C_LOC, chunk=3125, n_cores=N_CORES):
    cos_theta = np.ascontiguousarray(np.asarray(cos_theta), dtype=np.float32)
    target = np.asarray(target).astype(np.int64)
    nch = c_loc // chunk
    in_maps = []
    for i in range(n_cores):
        lo = i * c_loc
        sh = np.ascontiguousarray(cos_theta[:, lo : lo + c_loc]).reshape(-1)
        local = (target >= lo) & (target < lo + c_loc)
        li = np.where(local, target - lo, -1)  # [B] col within shard or -1
        # lcoff[p, k*nch+j] = li[k*128+p] - j*chunk if that lands in chunk j
        li_pk = li.reshape(NBLK, P).T  # [P, NBLK]
        lc = np.full((P, NBLK * nch), -1, dtype=np.int64)
        for j in range(nch):
            off = li_pk - j * chunk
            hit = (off >= 0) & (off < chunk) & (li_pk >= 0)
            lc[:, j::nch][hit] = off[hit]
        msk = np.ascontiguousarray(local.reshape(NBLK, P).T).astype(np.float32)
        in_maps.append(
            {
                "x": sh,
                "lcoff": np.ascontiguousarray(lc).astype(np.int32),
                "mask": msk,
            }
        )
    return in_maps


_CACHE = {}


def _get_nc():
    if "nc" not in _CACHE:
        _CACHE["nc"] = build()
    return _CACHE["nc"]


def run(cos_theta, target, trace=False):
    """Returns (loss ndarray shape (), exec_time_ns or None)."""
    nc = _get_nc()
    in_maps = prep_in_maps(cos_theta, target)
    res = bass_utils.run_bass_kernel_spmd(
        nc, in_maps, core_ids=list(range(N_CORES)), trace=trace
    )
    loss = np.asarray(res.results[0]["out"], dtype=np.float32).reshape(())
    return loss, res.exec_time_ns


def kernel(cos_theta, target):
    loss, _ = run(cos_theta, target)
    return loss
